# revision 1
# baseline (speedup 1.0000x reference)
"""Trainium2 Bass kernel for the 2-layer GATv2 + MLP-head model (nn_GAT_21028159881586).

Strategy (8 NeuronCores, SPMD single NEFF):
  * Destination-block partitioning: global nodes are split into 8 slices of
    3750 (padded to 3840 = 30 windows x 128 per core).  Core c owns all edges
    whose destination lands in its slice, so segment softmax + aggregation are
    core-local (no cross-core reduction of per-destination max/sum needed).
  * Per layer: data-parallel node transforms xl = x@Wl+bl / xr = x@Wr+br on
    the local slice, AllGather of xl (the table gathered by edge source) over
    the 8 cores, then 30 windows of 128 destinations each:
      - dma_gather of xl rows (src) in (channel, edge) and (edge, channel)
        layouts, and xr rows (dst) in (channel, edge) layout,
      - m = xl_src + xr_dst, leaky-relu (ACT Prelu alpha=0.2),
      - per-128-edge tile: attention logits via PE matmul against the
        per-head att vectors, exp on ACT (with a -30000 bias on padding edges),
        a 0/1 scatter matrix S built by DVE compare-vs-iota, and one PE matmul
        with rhs [exp*xl_src | exp] accumulating both the aggregation and the
        softmax denominators into PSUM,
      - normalize by 1/den, add bias, ELU, write the 128 output rows.
  * Softmax max-subtraction is skipped (logits are O(1); exp cannot overflow;
    mathematically identical).
  * MLP head: batch rows are assigned to the core owning their var node, the
    selected h2 rows are dma_gathered transposed, and the 3-layer MLP runs
    fully transposed (activations kept as (features, batch)).

Everything runs in fp16 with fp32 PSUM accumulation.
"""

import numpy as np

import concourse.bacc as bacc
import concourse.tile as tile
import concourse.mybir as mybir
from concourse.bass_utils import run_bass_kernel_spmd

P = 128
NCORES = 8
N = 30000
NLOC_REAL = 3750          # real nodes per core
WIN = 30                  # destination windows per core
NLOC = WIN * P            # 3840 padded nodes per core
NALL = NCORES * NLOC      # 30720 padded global nodes
IN_DIM = 1281
KCH = 11                  # input-dim chunks of 128
KPAD = KCH * P            # 1408
HID = 256
HEADS1 = 4
BLOC = 640                # padded batch rows per core (actual max ~554)
NEG = 0.2
PAD_BIAS = -30000.0

f32 = mybir.dt.float32
f16 = mybir.dt.float16
i16 = mybir.dt.int16
AF = mybir.ActivationFunctionType
OP = mybir.AluOpType

_nc_cache = {}


def _wrap16(idx2d: np.ndarray) -> np.ndarray:
    """(W, E) int -> (W*128, E//16) int16, wrapped in 16 partitions, replicated
    across the 8 gpsimd cores."""
    w, e = idx2d.shape
    assert e % 16 == 0
    t = idx2d.reshape(w, e // 16, 16).transpose(0, 2, 1)       # (W, 16, E/16)
    return np.tile(t, (1, 8, 1)).reshape(w * P, e // 16).astype(np.int16)


def _etile(v2d: np.ndarray) -> np.ndarray:
    """(W, E) -> (W*128, T) with [w*128+p, t] = v[w, t*128+p] (the layout of a
    transpose=False gather / per-tile PSUM partitions)."""
    w, e = v2d.shape
    t = v2d.reshape(w, e // P, P).transpose(0, 2, 1)           # (W, 128, T)
    return t.reshape(w * P, e // P)


def _preprocess(inputs):
    x = np.asarray(inputs["x"], np.float32)
    ei = np.asarray(inputs["edge_index"]).astype(np.int64)
    var_idx = np.asarray(inputs["var_node_idx"]).astype(np.int64)
    wt = np.asarray(inputs["wt_onehot"], np.float32)
    mut = np.asarray(inputs["mut_onehot"], np.float32)

    src = np.concatenate([ei[0], np.arange(N, dtype=np.int64)])
    dst = np.concatenate([ei[1], np.arange(N, dtype=np.int64)])
    # padded global id of every source node
    src_pad = (src // NLOC_REAL) * NLOC + (src % NLOC_REAL)

    order = np.argsort(dst, kind="stable")
    src_pad = src_pad[order]
    dst_s = dst[order]

    core_of = dst_s // NLOC_REAL
    dloc = dst_s - core_of * NLOC_REAL                      # local dst 0..3749
    win_of = dloc // P

    # max edges in any (core, window) -> uniform padded window size
    flat = core_of * WIN + win_of
    counts = np.bincount(flat, minlength=NCORES * WIN)
    # multiple of 896 so chunked dma_gathers (max safe ~896 idxs) write
    # contiguous regions
    ew = int(((counts.max() + 895) // 896) * 896)

    per_core = []
    for c in range(NCORES):
        sel = core_of == c
        sp_c, dl_c, w_c = src_pad[sel], dloc[sel], win_of[sel]
        srcw = np.zeros((WIN, ew), np.int64)
        dstw = np.zeros((WIN, ew), np.int64)
        drel = np.zeros((WIN, ew), np.float32)
        ebia = np.full((WIN, ew), PAD_BIAS, np.float32)
        for w in range(WIN):
            m = w_c == w
            k = int(m.sum())
            # order the window's edges by source for HBM locality in the xl
            # gathers (the S-matrix scatter handles any order; xr stays within
            # a 64KB region either way)
            o = np.argsort(sp_c[m], kind="stable")
            srcw[w, :k] = sp_c[m][o]
            dstw[w, :k] = dl_c[m][o]
            drel[w, :k] = (dl_c[m][o] - w * P).astype(np.float32)
            ebia[w, :k] = 0.0
        # pack per-window metadata into one u8 blob per row-block:
        # [srcidx i16 | dstloc i16 | dstrel f32 | ebias f32]
        si = _wrap16(srcw)                       # (WIN*P, ew//16) i16
        di = _wrap16(dstw)
        dr_ = _etile(drel).astype(np.float32)    # (WIN*P, T)
        eb_ = _etile(ebia).astype(np.float32)
        meta = np.concatenate([
            si.view(np.uint8).reshape(WIN * P, -1),
            di.view(np.uint8).reshape(WIN * P, -1),
            dr_.view(np.uint8).reshape(WIN * P, -1),
            eb_.view(np.uint8).reshape(WIN * P, -1)], axis=1)
        per_core.append(dict(meta=meta))

    # ---- shared weights / constants
    def pad_kT(w, m):  # (IN_DIM, m) -> (128, KCH*m) f16 chunked layout
        wp = np.zeros((KPAD, m), np.float32)
        wp[:IN_DIM] = w
        return wp.reshape(KCH, P, m).transpose(1, 0, 2).reshape(P, KCH * m).astype(np.float16)

    def two_chunk(w):  # (256, M) -> (128, 2*M) f16
        m = w.shape[1]
        return w.reshape(2, P, m).transpose(1, 0, 2).reshape(P, 2 * m).astype(np.float16)

    att1 = np.asarray(inputs["att1"], np.float32)           # (4, 64)
    att1_mat = np.zeros((HID, HEADS1), np.float32)
    for h in range(HEADS1):
        att1_mat[h * 64:(h + 1) * 64, h] = att1[h]
    att2_mat = np.asarray(inputs["att2"], np.float32).T     # (256, 1)

    def rep_bias(b):  # (HID,) -> (128, HID) f32
        return np.broadcast_to(np.asarray(b, np.float32)[None, :], (P, HID)).copy()

    hW1 = np.asarray(inputs["hW1"], np.float32)             # (296, 128)
    wlr1 = np.concatenate([np.asarray(inputs["Wl1"], np.float32),
                           np.asarray(inputs["Wr1"], np.float32)], axis=1)
    wlr2 = np.concatenate([np.asarray(inputs["Wl2"], np.float32),
                           np.asarray(inputs["Wr2"], np.float32)], axis=1)
    shared = dict(
        wlr1=pad_kT(wlr1, 2 * HID),
        wlr2=two_chunk(wlr2),
        att1=two_chunk(att1_mat),                            # (128, 2*4)
        att2=two_chunk(att2_mat),                            # (128, 2*1)
        blr1=np.concatenate([rep_bias(inputs["bl1"]), rep_bias(inputs["br1"])], 1),
        bias1=rep_bias(inputs["bias1"]),
        blr2=np.concatenate([rep_bias(inputs["bl2"]), rep_bias(inputs["br2"])], 1),
        bias2=rep_bias(inputs["bias2"]),
        hw1a=hW1[0:128].astype(np.float16),
        hw1b=hW1[128:256].astype(np.float16),
        hw1c=np.vstack([hW1[256:296], np.zeros((8, 128), np.float32)]).astype(np.float16),
        hw2=np.asarray(inputs["hW2"], np.float32).astype(np.float16),   # (128, 64)
        hw3=np.asarray(inputs["hW3"], np.float32).astype(np.float16),   # (64, 1)
        hb1=np.asarray(inputs["hb1"], np.float32).reshape(P, 1),
        hb2=np.asarray(inputs["hb2"], np.float32).reshape(64, 1),
        hb3=np.asarray(inputs["hb3"], np.float32).reshape(1, 1),
        iota=np.broadcast_to(np.arange(P, dtype=np.float32)[None, :], (P, P)).copy(),
    )

    # ---- per-core x slices, transposed + padded, chunked layout (128, KCH*NLOC)
    for c in range(NCORES):
        xp = np.zeros((KPAD, NLOC), np.float32)
        xp[:IN_DIM, :NLOC_REAL] = x[c * NLOC_REAL:(c + 1) * NLOC_REAL].T
        per_core[c]["xt"] = xp.reshape(KCH, P, NLOC).transpose(1, 0, 2).reshape(
            P, KCH * NLOC).astype(np.float16)

    # ---- MLP batch assignment: rows go to the core owning their var node
    vcore = var_idx // NLOC_REAL
    vloc = var_idx - vcore * NLOC_REAL
    batch_rows = []
    for c in range(NCORES):
        rows = np.nonzero(vcore == c)[0]
        assert len(rows) <= BLOC, f"core {c} has {len(rows)} batch rows > {BLOC}"
        batch_rows.append(rows)
        vi = np.zeros((1, BLOC), np.int64)
        vi[0, :len(rows)] = vloc[rows]
        per_core[c]["varloc"] = _wrap16(vi)
        wm = np.zeros((40, BLOC), np.float32)
        wm[:20, :len(rows)] = wt[rows].T
        wm[20:, :len(rows)] = mut[rows].T
        per_core[c]["wtmut"] = wm.astype(np.float16)

    return per_core, shared, batch_rows, ew


def _build(ew, sim_compat=False, no_collectives=False):
    T = ew // P
    nc = bacc.Bacc("TRN2", target_bir_lowering=False, debug=False,
                   num_devices=1 if no_collectives else NCORES,
                   num_swdge_queues=1)

    # ---------- I/O ----------
    mb = 2 * (ew // 16) * 2 + 2 * T * 4      # meta bytes per partition row
    io = {}
    io["xt"] = nc.dram_tensor("xt", [P, KCH * NLOC], f16, kind="ExternalInput")
    for nm, sh, dt in (
        ("wlr1", [P, KCH * 2 * HID], f16), ("wlr2", [P, 4 * HID], f16),
        ("att1", [P, 2 * HEADS1], f16), ("att2", [P, 2], f16),
        ("blr1", [P, 2 * HID], f32), ("bias1", [P, HID], f32),
        ("blr2", [P, 2 * HID], f32), ("bias2", [P, HID], f32),
        ("hw1a", [P, P], f16), ("hw1b", [P, P], f16), ("hw1c", [48, P], f16),
        ("hw2", [P, 64], f16), ("hw3", [64, 1], f16),
        ("hb1", [P, 1], f32), ("hb2", [64, 1], f32), ("hb3", [1, 1], f32),
        ("iota", [P, P], f32),
        ("meta", [WIN * P, mb], mybir.dt.uint8),
        ("varloc", [P, BLOC // 16], i16), ("wtmut", [40, BLOC], f16),
    ):
        io[nm] = nc.dram_tensor(nm, sh, dt, kind="ExternalInput")
    out = nc.dram_tensor("out", [1, BLOC], f32, kind="ExternalOutput")

    with tile.TileContext(nc) as tc:
        with (
            tc.tile_pool(name="const", bufs=1) as cp,
            tc.tile_pool(name="dram", bufs=1, space="DRAM") as dr,
        ):
            # resident constants
            c_ = {}
            for nm in ("wlr2", "att1", "att2", "bias1", "blr2", "bias2",
                       "hw1a", "hw1b", "hw1c", "hw2",
                       "hw3", "hb1", "hb2", "hb3", "iota", "varloc", "wtmut"):
                h = io[nm]
                c_[nm] = cp.tile(list(h.shape), h.dtype, tag=nm, name=f"c_{nm}")
                nc.sync.dma_start(c_[nm][:], h[:])

            # DRAM scratch
            xl1_loc = dr.tile([NLOC, HID], f16)
            xr1_loc = dr.tile([NLOC, HID], f16)
            xl1_all = dr.tile([NALL, HID], f16, addr_space="Shared")
            h1_loc = dr.tile([NLOC, HID], f16)
            xl2_loc = dr.tile([NLOC, HID], f16)
            xr2_loc = dr.tile([NLOC, HID], f16)
            xl2_all = dr.tile([NALL, HID], f16, addr_space="Shared")
            h2_loc = dr.tile([NLOC, HID], f16)

            # ---------- phase A layer 1 ----------
            with (
                tc.tile_pool(name="pa_sb", bufs=2) as sb,
                tc.tile_pool(name="pa_xt", bufs=1) as xp,
                tc.tile_pool(name="pa_ps", bufs=4, space="PSUM") as ps,
            ):
                xt = xp.tile([P, KCH, NLOC], f16)
                nc.sync.dma_start(xt[:], io["xt"][:].rearrange("p (k n) -> p k n", k=KCH))
                wlr1 = xp.tile([P, KCH, 2 * HID], f16)
                nc.sync.dma_start(wlr1[:], io["wlr1"][:].rearrange("p (k n) -> p k n", k=KCH))
                blr1 = xp.tile([P, 2 * HID], f32)
                nc.sync.dma_start(blr1[:], io["blr1"][:])
                for nt in range(WIN):
                    pa = ps.tile([P, 2 * HID], f32, tag="pa")
                    for k in range(KCH):
                        nc.tensor.matmul(pa[:], lhsT=xt[:, k, nt * P:(nt + 1) * P],
                                         rhs=wlr1[:, k, :],
                                         start=(k == 0), stop=(k == KCH - 1))
                    o = sb.tile([P, 2 * HID], f16, tag="pao")
                    nc.vector.tensor_tensor(out=o[:], in0=pa[:], in1=blr1[:],
                                            op=OP.add)
                    nc.scalar.dma_start(xl1_loc[nt * P:(nt + 1) * P, :], o[:, 0:HID])
                    nc.scalar.dma_start(xr1_loc[nt * P:(nt + 1) * P, :], o[:, HID:2 * HID])

            if no_collectives:
                nc.sync.dma_start(xl1_all[0:NLOC, :], xl1_loc[:])
            else:
                nc.gpsimd.collective_compute(
                    "AllGather", OP.bypass, replica_groups=[list(range(NCORES))],
                    ins=[xl1_loc[:].opt()], outs=[xl1_all[:].opt()])

            # ---------- layer 1 message passing ----------
            _emit_layer(nc, tc, ew=ew, heads=HEADS1, xl_all=xl1_all,
                        xr_loc=xr1_loc, h_out=h1_loc, att=c_["att1"],
                        bias_mat=c_["bias1"], io=io, iota=c_["iota"], tag="l1",
                        sim_compat=sim_compat)

            # ---------- phase A layer 2 ----------
            with (
                tc.tile_pool(name="pb_sb", bufs=2) as sb,
                tc.tile_pool(name="pb_ht", bufs=1) as hp,
                tc.tile_pool(name="pb_ps", bufs=4, space="PSUM") as ps,
            ):
                ht = hp.tile([P, 2, NLOC], f16)
                for k in range(2):
                    nc.sync.dma_start_transpose(ht[:, k, :],
                                                h1_loc[:, k * P:(k + 1) * P])
                blr2 = c_["blr2"]
                for nt in range(WIN):
                    pa = ps.tile([P, 2 * HID], f32, tag="pb")
                    for k in range(2):
                        nc.tensor.matmul(
                            pa[:], lhsT=ht[:, k, nt * P:(nt + 1) * P],
                            rhs=c_["wlr2"][:, k * 2 * HID:(k + 1) * 2 * HID],
                            start=(k == 0), stop=(k == 1))
                    o = sb.tile([P, 2 * HID], f16, tag="pbo")
                    nc.vector.tensor_tensor(out=o[:], in0=pa[:], in1=blr2[:],
                                            op=OP.add)
                    nc.scalar.dma_start(xl2_loc[nt * P:(nt + 1) * P, :], o[:, 0:HID])
                    nc.scalar.dma_start(xr2_loc[nt * P:(nt + 1) * P, :], o[:, HID:2 * HID])

            if no_collectives:
                nc.sync.dma_start(xl2_all[0:NLOC, :], xl2_loc[:])
            else:
                nc.gpsimd.collective_compute(
                    "AllGather", OP.bypass, replica_groups=[list(range(NCORES))],
                    ins=[xl2_loc[:].opt()], outs=[xl2_all[:].opt()])

            # ---------- layer 2 message passing ----------
            _emit_layer(nc, tc, ew=ew, heads=1, xl_all=xl2_all,
                        xr_loc=xr2_loc, h_out=h2_loc, att=c_["att2"],
                        bias_mat=c_["bias2"], io=io, iota=c_["iota"], tag="l2",
                        sim_compat=sim_compat)

            # ---------- MLP head ----------
            with (
                tc.tile_pool(name="mlp_sb", bufs=2) as sb,
                tc.tile_pool(name="mlp_ps", bufs=2, space="PSUM") as ps,
            ):
                sel = sb.tile([P, 2, BLOC], f16)
                nc.gpsimd.dma_gather(sel[:], h2_loc[:], c_["varloc"][:],
                                     num_idxs=BLOC, num_idxs_reg=BLOC,
                                     elem_size=HID, transpose=True)
                for c0, cn in ((0, 512), (512, BLOC - 512)):
                    z1p = ps.tile([P, 512], f32, tag="z1p")
                    nc.tensor.matmul(z1p[:, :cn], lhsT=c_["hw1a"][:],
                                     rhs=sel[:, 0, c0:c0 + cn], start=True, stop=False)
                    nc.tensor.matmul(z1p[:, :cn], lhsT=c_["hw1b"][:],
                                     rhs=sel[:, 1, c0:c0 + cn], start=False, stop=False)
                    nc.tensor.matmul(z1p[:, :cn], lhsT=c_["hw1c"][0:40, :],
                                     rhs=c_["wtmut"][:, c0:c0 + cn], start=False, stop=True)
                    z1 = sb.tile([P, 512], f16, tag="z1")
                    nc.scalar.activation(z1[:, :cn], z1p[:, :cn], AF.Relu,
                                         bias=c_["hb1"][:])
                    z2p = ps.tile([64, 512], f32, tag="z2p")
                    nc.tensor.matmul(z2p[:, :cn], lhsT=c_["hw2"][:],
                                     rhs=z1[:, :cn], start=True, stop=True)
                    z2 = sb.tile([64, 512], f16, tag="z2")
                    nc.scalar.activation(z2[:, :cn], z2p[:, :cn], AF.Relu,
                                         bias=c_["hb2"][:])
                    z3p = ps.tile([1, 512], f32, tag="z3p")
                    nc.tensor.matmul(z3p[:, :cn], lhsT=c_["hw3"][:],
                                     rhs=z2[:, :cn], start=True, stop=True)
                    z3 = sb.tile([1, 512], f32, tag="z3")
                    nc.scalar.activation(z3[:, :cn], z3p[:, :cn], AF.Identity,
                                         bias=c_["hb3"][:])
                    nc.sync.dma_start(out[0:1, c0:c0 + cn], z3[:, :cn])

    nc.compile()
    return nc


def _emit_layer(nc, tc, *, ew, heads, xl_all, xr_loc, h_out, att, bias_mat,
                io, iota, tag, sim_compat=False):
    T = ew // P
    CW = HID // heads
    NCHK = ew // 896            # chunks of 896 idxs (7 tiles of 128)
    i16b = 2 * (ew // 16)       # meta byte offsets
    mb = 2 * i16b + 2 * T * 4
    r896 = nc.gpsimd.to_reg(896)
    with (
        tc.tile_pool(name=f"{tag}_sb", bufs=2) as sb,
        tc.tile_pool(name=f"{tag}_sm", bufs=3) as sm,
        tc.tile_pool(name=f"{tag}_ps", bufs=2, space="PSUM") as ps,
        tc.tile_pool(name=f"{tag}_lg", bufs=4, space="PSUM") as lgp,
    ):
        for w in range(WIN):
            rows = slice(w * P, (w + 1) * P)
            meta = sm.tile([P, mb], mybir.dt.uint8, tag="meta")
            nc.sync.dma_start(meta[:], io["meta"][rows, :])
            si = meta[:, 0:i16b].bitcast(i16)
            di = meta[:, i16b:2 * i16b].bitcast(i16)
            drl = meta[:, 2 * i16b:2 * i16b + 4 * T].bitcast(f32)
            eb = meta[:, 2 * i16b + 4 * T:2 * i16b + 8 * T].bitcast(f32)

            g_ce = sb.tile([P, NCHK, 2, 896], f16, tag="g_ce")
            xr_ce = sb.tile([P, NCHK, 2, 896], f16, tag="xr_ce")
            g_ec = sb.tile([P, NCHK, 7, HID], f16, tag="g_ec")
            for c in range(NCHK):
                isl = slice(c * 56, (c + 1) * 56)
                nc.gpsimd.dma_gather(g_ce[:, c, :, :], xl_all[:], si[:, isl],
                                     num_idxs=896, num_idxs_reg=r896,
                                     elem_size=HID, transpose=True)
                nc.gpsimd.dma_gather(xr_ce[:, c, :, :], xr_loc[:], di[:, isl],
                                     num_idxs=896, num_idxs_reg=r896,
                                     elem_size=HID, transpose=True)
                nc.gpsimd.dma_gather(g_ec[:, c, :, :], xl_all[:], si[:, isl],
                                     num_idxs=896, num_idxs_reg=r896,
                                     elem_size=HID, transpose=False)

            m = sb.tile([P, NCHK, 2, 896], f16, tag="m")
            nc.vector.tensor_add(m[:], g_ce[:], xr_ce[:])
            lr = sb.tile([P, NCHK, 2, 896], f16, tag="lr")
            if sim_compat:
                # sim has no Prelu; max(x, 0.2x) is bit-identical on HW
                nc.vector.scalar_tensor_tensor(out=lr[:], in0=m[:], scalar=NEG,
                                               in1=m[:], op0=OP.mult, op1=OP.max)
            else:
                nc.scalar.activation(lr[:], m[:], AF.Prelu, alpha=NEG)

            aggden = ps.tile([P, HID + heads], f32, tag="aggden")
            for t in range(T):
                ck, r = t // 7, t % 7
                lg = lgp.tile([P, heads], f32, tag="lg")
                nc.tensor.matmul(lg[:], lhsT=lr[:, ck, 0, r * P:(r + 1) * P],
                                 rhs=att[:, 0:heads], start=True, stop=False)
                nc.tensor.matmul(lg[:], lhsT=lr[:, ck, 1, r * P:(r + 1) * P],
                                 rhs=att[:, heads:2 * heads],
                                 start=False, stop=True)
                gw = sb.tile([P, HID + heads], f16, tag="gw", bufs=4)
                nc.scalar.activation(gw[:, HID:HID + heads], lg[:], AF.Exp,
                                     bias=eb[:, t:t + 1])
                s_t = sb.tile([P, P], f16, tag="s_t", bufs=4)
                nc.vector.tensor_tensor(out=s_t[:],
                                        in0=drl[:, t:t + 1].to_broadcast([P, P]),
                                        in1=iota[:], op=OP.is_equal)
                nc.vector.tensor_tensor(
                    out=gw[:, 0:HID].rearrange("p (h c) -> p h c", h=heads),
                    in0=g_ec[:, ck, r, :].rearrange("p (h c) -> p h c", h=heads),
                    in1=gw[:, HID:HID + heads].to_broadcast([P, heads, CW]),
                    op=OP.mult)
                nc.tensor.matmul(aggden[:], lhsT=s_t[:], rhs=gw[:],
                                 start=(t == 0), stop=(t == T - 1))

            den = sb.tile([P, heads], f32, tag="den")
            nc.vector.tensor_scalar_add(den[:], aggden[:, HID:HID + heads], 1e-16)
            rden = sb.tile([P, heads], f32, tag="rden")
            nc.vector.reciprocal(rden[:], den[:])
            hn = sb.tile([P, HID], f32, tag="hn")
            nc.vector.tensor_tensor(
                out=hn[:].rearrange("p (h c) -> p h c", h=heads),
                in0=aggden[:, 0:HID].rearrange("p (h c) -> p h c", h=heads),
                in1=rden[:].to_broadcast([P, heads, CW]), op=OP.mult)
            hb = sb.tile([P, HID], f32, tag="hb")
            nc.vector.tensor_tensor(out=hb[:], in0=hn[:], in1=bias_mat[:], op=OP.add)
            # ELU(x) = relu(x) + exp(min(x, 0)) - 1
            mn = sb.tile([P, HID], f32, tag="mn")
            nc.vector.tensor_scalar_min(mn[:], hb[:], 0.0)
            ex = sb.tile([P, HID], f32, tag="ex")
            nc.scalar.activation(ex[:], mn[:], AF.Exp)
            el = sb.tile([P, HID], f32, tag="el")
            nc.vector.scalar_tensor_tensor(out=el[:], in0=hb[:], scalar=0.0,
                                           in1=ex[:], op0=OP.max, op1=OP.add)
            h_t = sb.tile([P, HID], f16, tag="h_t")
            nc.vector.tensor_scalar_add(h_t[:], el[:], -1.0)
            nc.scalar.dma_start(h_out[rows, :], h_t[:])


def kernel(**inputs):
    per_core, shared, batch_rows, ew = _preprocess(inputs)

    if ew not in _nc_cache:
        _nc_cache[ew] = _build(ew)
    nc = _nc_cache[ew]

    in_maps = []
    for c in range(NCORES):
        m = dict(shared)
        m.update(per_core[c])
        in_maps.append({k: np.ascontiguousarray(v) for k, v in m.items()})

    res = run_bass_kernel_spmd(nc, in_maps, core_ids=list(range(NCORES)))

    B = len(np.asarray(inputs["var_node_idx"]))
    out = np.zeros((B,), np.float32)
    for c in range(NCORES):
        rows = batch_rows[c]
        out[rows] = res.results[c]["out"][0, :len(rows)]
    return out



# revision 5
# speedup vs baseline: 1.4769x; 1.4769x over previous
"""Trainium2 Bass kernel for the 2-layer GATv2 + MLP-head model (nn_GAT_21028159881586).

Strategy (8 NeuronCores, SPMD single NEFF):
  * Destination-block partitioning: global nodes are split into 8 slices of
    3750 (padded to 3840 = 30 windows x 128 per core).  Core c owns all edges
    whose destination lands in its slice, so segment softmax + aggregation are
    core-local.
  * Per layer: data-parallel node transforms xl = x@Wl+bl / xr = x@Wr+br on
    the local slice; xr stays resident in SBUF, xl is AllGathered across the
    8 cores, then 30 windows of 128 destinations each are processed.
  * Per window (the V2 pipeline -- exactly ONE row-major gather stream):
      - dma_gather of xl rows (by edge source) in (edge, channel) layout only
        (3 chunks of 896 idxs).  The (channel, edge) copy and the xr[dst]
        gather of V1 are gone: xr[dst] is reconstructed on the PE from the
        128 window xr rows via the transposed 0/1 scatter matrix, and the
        logits contract over channels on the DVE instead of the PE.
      - S   [e,d] = (drl[e] == d)   built by one DVE compare  (agg lhsT)
        S^T [d,e]                   built by DVE compare against a partition-
        broadcast of drl (broadcast done by a stride-0 DMA or a K=1 matmul)
      - per tile t: PSUM m = S^T.T @ xr_window + I.T @ xl_src  (PE), then
        ACT Prelu(m) -> lr, DVE lr *= att (broadcast), DVE segmented reduce
        -> logits, += pad bias, ACT exp, DVE xl_src *= exp (in place), PE
        aggregation matmuls into PSUM [agg | den].
      - normalize by 1/den, add bias, ELU, write the 128 output rows.
  * Softmax max-subtraction is skipped (logits are O(1); exp cannot overflow).
  * MLP head: batch rows are assigned to the core owning their var node, the
    selected h2 rows are dma_gathered transposed, and the 3-layer MLP runs
    fully transposed.

Everything runs in fp16 with fp32 PSUM accumulation.
"""

import numpy as np

import concourse.bacc as bacc
import concourse.tile as tile
import concourse.mybir as mybir
from concourse.bass_utils import run_bass_kernel_spmd

P = 128
NCORES = 8
N = 30000
NLOC_REAL = 3750          # real nodes per core
WIN = 30                  # destination windows per core
NLOC = WIN * P            # 3840 padded nodes per core
NALL = NCORES * NLOC      # 30720 padded global nodes
IN_DIM = 1281
KCH = 11                  # input-dim chunks of 128
KPAD = KCH * P            # 1408
HID = 256
HEADS1 = 4
BLOC = 640                # padded batch rows per core (actual max ~554)
NEG = 0.2
PAD_BIAS = -30000.0
BATCH_M = 4               # tiles per PSUM m batch

f32 = mybir.dt.float32
f16 = mybir.dt.float16
i16 = mybir.dt.int16
AF = mybir.ActivationFunctionType
OP = mybir.AluOpType

USE_BCAST_DMA = True      # stride-0 partition broadcast via DMA for drl row

_nc_cache = {}


def _wrap16(idx2d: np.ndarray) -> np.ndarray:
    """(W, E) int -> (W*128, E//16) int16, wrapped in 16 partitions, replicated
    across the 8 gpsimd cores."""
    w, e = idx2d.shape
    assert e % 16 == 0
    t = idx2d.reshape(w, e // 16, 16).transpose(0, 2, 1)       # (W, 16, E/16)
    return np.tile(t, (1, 8, 1)).reshape(w * P, e // 16).astype(np.int16)


def _etile(v2d: np.ndarray) -> np.ndarray:
    """(W, E) -> (W*128, T) with [w*128+p, t] = v[w, t*128+p] (per-tile
    edge-partition layout)."""
    w, e = v2d.shape
    t = v2d.reshape(w, e // P, P).transpose(0, 2, 1)           # (W, 128, T)
    return t.reshape(w * P, e // P)


def _preprocess(inputs):
    x = np.asarray(inputs["x"], np.float32)
    ei = np.asarray(inputs["edge_index"]).astype(np.int64)
    var_idx = np.asarray(inputs["var_node_idx"]).astype(np.int64)
    wt = np.asarray(inputs["wt_onehot"], np.float32)
    mut = np.asarray(inputs["mut_onehot"], np.float32)

    src = np.concatenate([ei[0], np.arange(N, dtype=np.int64)])
    dst = np.concatenate([ei[1], np.arange(N, dtype=np.int64)])
    # padded global id of every source node
    src_pad = (src // NLOC_REAL) * NLOC + (src % NLOC_REAL)

    order = np.argsort(dst, kind="stable")
    src_pad = src_pad[order]
    dst_s = dst[order]

    core_of = dst_s // NLOC_REAL
    dloc = dst_s - core_of * NLOC_REAL                      # local dst 0..3749
    win_of = dloc // P

    flat = core_of * WIN + win_of
    counts = np.bincount(flat, minlength=NCORES * WIN)
    # multiple of 896 so chunked dma_gathers (max safe ~896 idxs) write
    # contiguous regions
    ew = int(((counts.max() + 895) // 896) * 896)

    per_core = []
    for c in range(NCORES):
        sel = core_of == c
        sp_c, dl_c, w_c = src_pad[sel], dloc[sel], win_of[sel]
        srcw = np.zeros((WIN, ew), np.int64)
        drel = np.zeros((WIN, ew), np.float32)
        ebia = np.full((WIN, ew), PAD_BIAS, np.float32)
        for w in range(WIN):
            m = w_c == w
            k = int(m.sum())
            # order the window's edges by source for HBM locality in the xl
            # gather
            o = np.argsort(sp_c[m], kind="stable")
            srcw[w, :k] = sp_c[m][o]
            drel[w, :k] = (dl_c[m][o] - w * P).astype(np.float32)
            ebia[w, :k] = 0.0
        # pack per-window metadata into one u8 blob per row-block:
        # [srcidx i16 | dstrel f16 | ebias f16]
        si = _wrap16(srcw)                         # (WIN*P, ew//16) i16
        dr_ = _etile(drel).astype(np.float16)      # (WIN*P, T)
        eb_ = _etile(ebia).astype(np.float16)
        meta = np.concatenate([
            si.view(np.uint8).reshape(WIN * P, -1),
            dr_.view(np.uint8).reshape(WIN * P, -1),
            eb_.view(np.uint8).reshape(WIN * P, -1)], axis=1)
        per_core.append(dict(meta=meta, drow=drel.astype(np.float16)))

    # ---- shared weights / constants
    def pad_kT(w, m):  # (IN_DIM, m) -> (128, KCH*m) f16 chunked layout
        wp = np.zeros((KPAD, m), np.float32)
        wp[:IN_DIM] = w
        return wp.reshape(KCH, P, m).transpose(1, 0, 2).reshape(P, KCH * m).astype(np.float16)

    def two_chunk(w):  # (256, M) -> (128, 2*M) f16
        m = w.shape[1]
        return w.reshape(2, P, m).transpose(1, 0, 2).reshape(P, 2 * m).astype(np.float16)

    att1 = np.asarray(inputs["att1"], np.float32)           # (4, 64)
    attb1 = np.broadcast_to(att1.reshape(1, HID), (P, HID)).copy()
    attb2 = np.broadcast_to(np.asarray(inputs["att2"], np.float32).reshape(1, HID),
                            (P, HID)).copy()

    def rep_bias(b):  # (HID,) -> (128, HID) f32
        return np.broadcast_to(np.asarray(b, np.float32)[None, :], (P, HID)).copy()

    hW1 = np.asarray(inputs["hW1"], np.float32)             # (296, 128)
    wlr1 = np.concatenate([np.asarray(inputs["Wl1"], np.float32),
                           np.asarray(inputs["Wr1"], np.float32)], axis=1)
    wlr2 = np.concatenate([np.asarray(inputs["Wl2"], np.float32),
                           np.asarray(inputs["Wr2"], np.float32)], axis=1)
    shared = dict(
        wlr1=pad_kT(wlr1, 2 * HID),
        wlr2=two_chunk(wlr2),
        attb1=attb1.astype(np.float16),
        attb2=attb2.astype(np.float16),
        blr1=np.concatenate([rep_bias(inputs["bl1"]), rep_bias(inputs["br1"])], 1),
        bias1=rep_bias(inputs["bias1"]),
        blr2=np.concatenate([rep_bias(inputs["bl2"]), rep_bias(inputs["br2"])], 1),
        bias2=rep_bias(inputs["bias2"]),
        hw1a=hW1[0:128].astype(np.float16),
        hw1b=hW1[128:256].astype(np.float16),
        hw1c=np.vstack([hW1[256:296], np.zeros((8, 128), np.float32)]).astype(np.float16),
        hw2=np.asarray(inputs["hW2"], np.float32).astype(np.float16),   # (128, 64)
        hw3=np.asarray(inputs["hW3"], np.float32).astype(np.float16),   # (64, 1)
        hb1=np.asarray(inputs["hb1"], np.float32).reshape(P, 1),
        hb2=np.asarray(inputs["hb2"], np.float32).reshape(64, 1),
        hb3=np.asarray(inputs["hb3"], np.float32).reshape(1, 1),
        iota=np.broadcast_to(np.arange(P, dtype=np.float16)[None, :], (P, P)).copy(),
        iotat=np.arange(P, dtype=np.float32).reshape(P, 1).copy(),
        ident=np.eye(P, dtype=np.float16),
        ones1=np.ones((1, P), np.float16),
    )

    # ---- per-core x slices, transposed + padded, chunked layout (128, KCH*NLOC)
    for c in range(NCORES):
        xp = np.zeros((KPAD, NLOC), np.float32)
        xp[:IN_DIM, :NLOC_REAL] = x[c * NLOC_REAL:(c + 1) * NLOC_REAL].T
        per_core[c]["xt"] = xp.reshape(KCH, P, NLOC).transpose(1, 0, 2).reshape(
            P, KCH * NLOC).astype(np.float16)

    # ---- MLP batch assignment: rows go to the core owning their var node
    vcore = var_idx // NLOC_REAL
    vloc = var_idx - vcore * NLOC_REAL
    batch_rows = []
    for c in range(NCORES):
        rows = np.nonzero(vcore == c)[0]
        assert len(rows) <= BLOC, f"core {c} has {len(rows)} batch rows > {BLOC}"
        batch_rows.append(rows)
        vi = np.zeros((1, BLOC), np.int64)
        vi[0, :len(rows)] = vloc[rows]
        per_core[c]["varloc"] = _wrap16(vi)
        wm = np.zeros((40, BLOC), np.float32)
        wm[:20, :len(rows)] = wt[rows].T
        wm[20:, :len(rows)] = mut[rows].T
        per_core[c]["wtmut"] = wm.astype(np.float16)

    return per_core, shared, batch_rows, ew


def _build(ew):
    T = ew // P
    nc = bacc.Bacc("TRN2", target_bir_lowering=False, debug=False,
                   num_devices=NCORES, num_swdge_queues=1)

    # ---------- I/O ----------
    mb = 2 * (ew // 16) + 2 * T + 2 * T      # meta bytes per partition row
    io = {}
    io["xt"] = nc.dram_tensor("xt", [P, KCH * NLOC], f16, kind="ExternalInput")
    for nm, sh, dt in (
        ("wlr1", [P, KCH * 2 * HID], f16), ("wlr2", [P, 4 * HID], f16),
        ("attb1", [P, HID], f16), ("attb2", [P, HID], f16),
        ("blr1", [P, 2 * HID], f32), ("bias1", [P, HID], f32),
        ("blr2", [P, 2 * HID], f32), ("bias2", [P, HID], f32),
        ("hw1a", [P, P], f16), ("hw1b", [P, P], f16), ("hw1c", [48, P], f16),
        ("hw2", [P, 64], f16), ("hw3", [64, 1], f16),
        ("hb1", [P, 1], f32), ("hb2", [64, 1], f32), ("hb3", [1, 1], f32),
        ("iota", [P, P], f16), ("iotat", [P, 1], f32),
        ("ident", [P, P], f16), ("ones1", [1, P], f16),
        ("meta", [WIN * P, mb], mybir.dt.uint8),
        ("drow", [WIN, ew], f16),
        ("varloc", [P, BLOC // 16], i16), ("wtmut", [40, BLOC], f16),
    ):
        io[nm] = nc.dram_tensor(nm, sh, dt, kind="ExternalInput")
    out = nc.dram_tensor("out", [1, BLOC], f32, kind="ExternalOutput")

    with tile.TileContext(nc) as tc:
        with (
            tc.tile_pool(name="const", bufs=1) as cp,
            tc.tile_pool(name="dram", bufs=1, space="DRAM") as dr,
        ):
            # resident constants
            c_ = {}
            for nm in ("wlr2", "attb1", "attb2", "bias1", "blr2", "bias2",
                       "hw1a", "hw1b", "hw1c", "hw2", "hw3", "hb1", "hb2",
                       "hb3", "iota", "iotat", "ident", "ones1",
                       "varloc", "wtmut"):
                h = io[nm]
                c_[nm] = cp.tile(list(h.shape), h.dtype, tag=nm, name=f"c_{nm}")
                nc.sync.dma_start(c_[nm][:], h[:])

            # DRAM scratch
            xl1_loc = dr.tile([NLOC, HID], f16)
            xl1_all = dr.tile([NALL, HID], f16, addr_space="Shared")
            h1_loc = dr.tile([NLOC, HID], f16)
            xl2_loc = dr.tile([NLOC, HID], f16)
            xl2_all = dr.tile([NALL, HID], f16, addr_space="Shared")
            h2_loc = dr.tile([NLOC, HID], f16)

            # ================= layer 1 =================
            with tc.tile_pool(name="l1_xr", bufs=1) as xrp:
                xr1 = xrp.tile([P, WIN, HID], f16)
                # ---------- phase A layer 1 ----------
                with (
                    tc.tile_pool(name="pa_sb", bufs=2) as sb,
                    tc.tile_pool(name="pa_xt", bufs=1) as xp,
                    tc.tile_pool(name="pa_ps", bufs=4, space="PSUM") as ps,
                ):
                    xt = xp.tile([P, KCH, NLOC], f16)
                    nc.sync.dma_start(xt[:], io["xt"][:].rearrange("p (k n) -> p k n", k=KCH))
                    wlr1 = xp.tile([P, KCH, 2 * HID], f16)
                    nc.sync.dma_start(wlr1[:], io["wlr1"][:].rearrange("p (k n) -> p k n", k=KCH))
                    blr1 = xp.tile([P, 2 * HID], f32)
                    nc.sync.dma_start(blr1[:], io["blr1"][:])
                    for nt in range(WIN):
                        pa = ps.tile([P, 2 * HID], f32, tag="pa")
                        for k in range(KCH):
                            nc.tensor.matmul(pa[:], lhsT=xt[:, k, nt * P:(nt + 1) * P],
                                             rhs=wlr1[:, k, :],
                                             start=(k == 0), stop=(k == KCH - 1))
                        o = sb.tile([P, HID], f16, tag="pao")
                        nc.vector.tensor_tensor(out=o[:], in0=pa[:, 0:HID],
                                                in1=blr1[:, 0:HID], op=OP.add)
                        nc.vector.tensor_tensor(out=xr1[:, nt, :], in0=pa[:, HID:2 * HID],
                                                in1=blr1[:, HID:2 * HID], op=OP.add)
                        nc.scalar.dma_start(xl1_loc[nt * P:(nt + 1) * P, :], o[:])

                nc.gpsimd.collective_compute(
                    "AllGather", OP.bypass, replica_groups=[list(range(NCORES))],
                    ins=[xl1_loc[:].opt()], outs=[xl1_all[:].opt()])

                _emit_mp(nc, tc, ew=ew, heads=HEADS1, xl_all=xl1_all,
                         xr_sb=xr1, h_out=h1_loc, attb=c_["attb1"],
                         bias_mat=c_["bias1"], io=io, c_=c_, tag="l1")

            # ================= layer 2 =================
            with tc.tile_pool(name="l2_xr", bufs=1) as xrp:
                xr2 = xrp.tile([P, WIN, HID], f16)
                with (
                    tc.tile_pool(name="pb_sb", bufs=2) as sb,
                    tc.tile_pool(name="pb_ht", bufs=1) as hp,
                    tc.tile_pool(name="pb_ps", bufs=4, space="PSUM") as ps,
                ):
                    ht = hp.tile([P, 2, NLOC], f16)
                    for k in range(2):
                        nc.sync.dma_start_transpose(ht[:, k, :],
                                                    h1_loc[:, k * P:(k + 1) * P])
                    blr2 = c_["blr2"]
                    for nt in range(WIN):
                        pa = ps.tile([P, 2 * HID], f32, tag="pb")
                        for k in range(2):
                            nc.tensor.matmul(
                                pa[:], lhsT=ht[:, k, nt * P:(nt + 1) * P],
                                rhs=c_["wlr2"][:, k * 2 * HID:(k + 1) * 2 * HID],
                                start=(k == 0), stop=(k == 1))
                        o = sb.tile([P, HID], f16, tag="pbo")
                        nc.vector.tensor_tensor(out=o[:], in0=pa[:, 0:HID],
                                                in1=blr2[:, 0:HID], op=OP.add)
                        nc.vector.tensor_tensor(out=xr2[:, nt, :], in0=pa[:, HID:2 * HID],
                                                in1=blr2[:, HID:2 * HID], op=OP.add)
                        nc.scalar.dma_start(xl2_loc[nt * P:(nt + 1) * P, :], o[:])

                nc.gpsimd.collective_compute(
                    "AllGather", OP.bypass, replica_groups=[list(range(NCORES))],
                    ins=[xl2_loc[:].opt()], outs=[xl2_all[:].opt()])

                _emit_mp(nc, tc, ew=ew, heads=1, xl_all=xl2_all,
                         xr_sb=xr2, h_out=h2_loc, attb=c_["attb2"],
                         bias_mat=c_["bias2"], io=io, c_=c_, tag="l2")

            # ---------- MLP head ----------
            with (
                tc.tile_pool(name="mlp_sb", bufs=2) as sb,
                tc.tile_pool(name="mlp_ps", bufs=2, space="PSUM") as ps,
            ):
                sel = sb.tile([P, 2, BLOC], f16)
                nc.gpsimd.dma_gather(sel[:], h2_loc[:], c_["varloc"][:],
                                     num_idxs=BLOC, num_idxs_reg=BLOC,
                                     elem_size=HID, transpose=True)
                for c0, cn in ((0, 512), (512, BLOC - 512)):
                    z1p = ps.tile([P, 512], f32, tag="z1p")
                    nc.tensor.matmul(z1p[:, :cn], lhsT=c_["hw1a"][:],
                                     rhs=sel[:, 0, c0:c0 + cn], start=True, stop=False)
                    nc.tensor.matmul(z1p[:, :cn], lhsT=c_["hw1b"][:],
                                     rhs=sel[:, 1, c0:c0 + cn], start=False, stop=False)
                    nc.tensor.matmul(z1p[:, :cn], lhsT=c_["hw1c"][0:40, :],
                                     rhs=c_["wtmut"][:, c0:c0 + cn], start=False, stop=True)
                    z1 = sb.tile([P, 512], f16, tag="z1")
                    nc.scalar.activation(z1[:, :cn], z1p[:, :cn], AF.Relu,
                                         bias=c_["hb1"][:])
                    z2p = ps.tile([64, 512], f32, tag="z2p")
                    nc.tensor.matmul(z2p[:, :cn], lhsT=c_["hw2"][:],
                                     rhs=z1[:, :cn], start=True, stop=True)
                    z2 = sb.tile([64, 512], f16, tag="z2")
                    nc.scalar.activation(z2[:, :cn], z2p[:, :cn], AF.Relu,
                                         bias=c_["hb2"][:])
                    z3p = ps.tile([1, 512], f32, tag="z3p")
                    nc.tensor.matmul(z3p[:, :cn], lhsT=c_["hw3"][:],
                                     rhs=z2[:, :cn], start=True, stop=True)
                    z3 = sb.tile([1, 512], f32, tag="z3")
                    nc.scalar.activation(z3[:, :cn], z3p[:, :cn], AF.Identity,
                                         bias=c_["hb3"][:])
                    nc.sync.dma_start(out[0:1, c0:c0 + cn], z3[:, :cn])

    nc.compile()
    return nc


def _emit_mp(nc, tc, *, ew, heads, xl_all, xr_sb, h_out, attb, bias_mat,
             io, c_, tag):
    """Message passing for one GATv2 layer (V2 pipeline)."""
    T = ew // P
    CW = HID // heads
    NCHK = ew // 896
    i16b = 2 * (ew // 16)
    mb = i16b + 2 * T + 2 * T
    NB = (T + BATCH_M - 1) // BATCH_M
    r896 = nc.gpsimd.to_reg(896)
    with (
        tc.tile_pool(name=f"{tag}_g", bufs=2) as gp,
        tc.tile_pool(name=f"{tag}_sb", bufs=2) as sb,
        tc.tile_pool(name=f"{tag}_sm", bufs=3) as sm,
        tc.tile_pool(name=f"{tag}_pm", bufs=2, space="PSUM") as pmp,
        tc.tile_pool(name=f"{tag}_pa", bufs=2, space="PSUM") as pap,
    ):
        for w in range(WIN):
            rows = slice(w * P, (w + 1) * P)
            meta = sm.tile([P, mb], mybir.dt.uint8, tag="meta")
            nc.sync.dma_start(meta[:], io["meta"][rows, :])
            si = meta[:, 0:i16b].bitcast(i16)
            drl = meta[:, i16b:i16b + 2 * T].bitcast(f16)
            eb = meta[:, i16b + 2 * T:i16b + 4 * T].bitcast(f16)

            # ---- gather xl rows by source (row-major), 3 chunks of 896
            g = gp.tile([P, T, HID], f16, tag="g")
            for c in range(NCHK):
                nc.gpsimd.dma_gather(g[:, c * 7:(c + 1) * 7, :], xl_all[:],
                                     si[:, c * 56:(c + 1) * 56],
                                     num_idxs=896, num_idxs_reg=r896,
                                     elem_size=HID, transpose=False)

            # ---- S [e, (t,d)] = (drl[e,t] == d)
            S = sb.tile([P, T, P], f16, tag="S")
            nc.vector.tensor_tensor(
                out=S[:],
                in0=drl[:].rearrange("p (t o) -> p t o", o=1).to_broadcast([P, T, P]),
                in1=c_["iota"][:].rearrange("p (o d) -> p o d", o=1).to_broadcast([P, T, P]),
                op=OP.is_equal)

            # ---- S^T [d, (t,e)] = (d == drl[e,t]) via partition-broadcast
            sT = sb.tile([P, T, P], f16, tag="sT")
            if USE_BCAST_DMA:
                db = sb.tile([P, ew], f16, tag="db")
                nc.sync.dma_start(db[:], io["drow"][w:w + 1, :].to_broadcast([P, ew]))
                nc.vector.tensor_tensor(
                    out=sT[:].rearrange("p t e -> p (t e)"),
                    in0=c_["iotat"][:].to_broadcast([P, ew]),
                    in1=db[:], op=OP.is_equal)
            else:
                drs = sb.tile([1, ew], f16, tag="drs")
                nc.sync.dma_start(drs[:], io["drow"][w:w + 1, :])
                for c0 in range(0, ew, 512):
                    cn = min(512, ew - c0)
                    pb = pmp.tile([P, 512], f32, tag="pb")
                    nc.tensor.matmul(pb[:, :cn], lhsT=c_["ones1"][:],
                                     rhs=drs[:, c0:c0 + cn], start=True, stop=True)
                    nc.vector.tensor_tensor(
                        out=sT[:].rearrange("p t e -> p (t e)")[:, c0:c0 + cn],
                        in0=c_["iotat"][:].to_broadcast([P, cn]),
                        in1=pb[:, :cn], op=OP.is_equal)

            # ---- v = S^T.T @ xr_win + I.T @ g  (PE), prelu -> lr (ACT)
            lr = sb.tile([P, T, HID], f16, tag="lr")
            for b in range(NB):
                t0 = b * BATCH_M
                t1 = min(t0 + BATCH_M, T)
                pm = pmp.tile([P, BATCH_M, HID], f32, tag="pm")
                # keep each slot's 2-matmul accumulation group consecutive: a
                # start=True wipes the whole bank's has_written bits, so
                # interleaving open groups that share a bank drops terms
                for t in range(t0, t1):
                    nc.tensor.matmul(pm[:, t - t0, :], lhsT=sT[:, t, :],
                                     rhs=xr_sb[:, w, :], start=True, stop=False)
                    nc.tensor.matmul(pm[:, t - t0, :], lhsT=c_["ident"][:],
                                     rhs=g[:, t, :], start=False, stop=True)
                nc.scalar.activation(lr[:, t0:t1, :], pm[:, 0:t1 - t0, :],
                                     AF.Prelu, alpha=NEG)

            # ---- logits: lr *= att (broadcast), segmented reduce, +pad bias
            nc.vector.tensor_tensor(
                out=lr[:], in0=lr[:],
                in1=attb[:].rearrange("p (o c) -> p o c", o=1).to_broadcast([P, T, HID]),
                op=OP.mult)
            lg = sb.tile([P, T, heads], f32, tag="lg")
            nc.vector.tensor_reduce(
                out=lg[:].rearrange("p t h -> p (t h)"),
                in_=lr[:].rearrange("p t (h c) -> p (t h) c", h=heads),
                axis=mybir.AxisListType.X, op=OP.add)
            nc.vector.tensor_tensor(
                out=lg[:], in0=lg[:],
                in1=eb[:].rearrange("p (t o) -> p t o", o=1).to_broadcast([P, T, heads]),
                op=OP.add)
            ex = sb.tile([P, T, heads], f16, tag="ex")
            nc.scalar.activation(ex[:], lg[:], AF.Exp)

            # ---- g *= exp (in place), aggregate on PE
            nc.vector.tensor_tensor(
                out=g[:].rearrange("p t (h c) -> p t h c", h=heads),
                in0=g[:].rearrange("p t (h c) -> p t h c", h=heads),
                in1=ex[:].rearrange("p t (h o) -> p t h o", o=1).to_broadcast([P, T, heads, CW]),
                op=OP.mult)
            # agg and den must live in DIFFERENT banks: both groups stay
            # open across all T tiles, and a start=True in a shared bank
            # would wipe the other group's has_written bits
            ad = pap.tile([P, 512], f32, tag="ad")      # full bank
            dn = pap.tile([P, heads], f32, tag="dn")    # separate bank
            for t in range(T):
                nc.tensor.matmul(ad[:, 0:HID], lhsT=S[:, t, :], rhs=g[:, t, :],
                                 start=(t == 0), stop=(t == T - 1))
                nc.tensor.matmul(dn[:], lhsT=S[:, t, :],
                                 rhs=ex[:, t, :], start=(t == 0), stop=(t == T - 1))

            # ---- normalize + bias + ELU
            den = sb.tile([P, heads], f32, tag="den")
            nc.vector.tensor_scalar_add(den[:], dn[:], 1e-16)
            rden = sb.tile([P, heads], f32, tag="rden")
            nc.vector.reciprocal(rden[:], den[:])
            hn = sb.tile([P, HID], f32, tag="hn")
            nc.vector.tensor_tensor(
                out=hn[:].rearrange("p (h c) -> p h c", h=heads),
                in0=ad[:, 0:HID].rearrange("p (h c) -> p h c", h=heads),
                in1=rden[:].rearrange("p (h o) -> p h o", o=1).to_broadcast([P, heads, CW]),
                op=OP.mult)
            hb = sb.tile([P, HID], f32, tag="hb")
            nc.vector.tensor_tensor(out=hb[:], in0=hn[:], in1=bias_mat[:], op=OP.add)
            # ELU(x) = relu(x) + exp(min(x, 0)) - 1
            mn = sb.tile([P, HID], f32, tag="mn")
            nc.vector.tensor_scalar_min(mn[:], hb[:], 0.0)
            exe = sb.tile([P, HID], f32, tag="exe")
            nc.scalar.activation(exe[:], mn[:], AF.Exp)
            el = sb.tile([P, HID], f32, tag="el")
            nc.vector.scalar_tensor_tensor(out=el[:], in0=hb[:], scalar=0.0,
                                           in1=exe[:], op0=OP.max, op1=OP.add)
            h_t = sb.tile([P, HID], f16, tag="h_t")
            nc.vector.tensor_scalar_add(h_t[:], el[:], -1.0)
            nc.scalar.dma_start(h_out[rows, :], h_t[:])


def kernel(**inputs):
    per_core, shared, batch_rows, ew = _preprocess(inputs)

    if ew not in _nc_cache:
        _nc_cache[ew] = _build(ew)
    nc = _nc_cache[ew]

    in_maps = []
    for c in range(NCORES):
        m = dict(shared)
        m.update(per_core[c])
        in_maps.append({k: np.ascontiguousarray(v) for k, v in m.items()})

    res = run_bass_kernel_spmd(nc, in_maps, core_ids=list(range(NCORES)))

    B = len(np.asarray(inputs["var_node_idx"]))
    out = np.zeros((B,), np.float32)
    for c in range(NCORES):
        rows = batch_rows[c]
        out[rows] = res.results[c]["out"][0, :len(rows)]
    return out
